# revision 14
# baseline (speedup 1.0000x reference)
"""BERT encoder forward pass on 8 TRN2 NeuronCores.

Strategy: pure data parallelism over the batch (16 sequences -> 2 per core).
Each core runs the full 12-layer encoder on its 2 sequences; no collectives.

Per-core layout (T = 2*512 = 1024 tokens, L = 512 per sequence):
  h_t  : residual accumulator, fp32, token-major      [t(8x128 part), d(768)]
  hT_s : per-sequence transposed activations, bf16    [d(128 part), dchunk(6), t(512)]
         refreshed per layer by per-chunk SBUF->SBUF xbar DMA transposes
         (scalar queue carries ONLY transposes -> no xbar-mode flips).
         Attention context overwrites hT_s in place (WAR-tracked).
  qp/kp: q^T / k^T per (seq, head-pair), bf16         [128 part = 2 heads x 64dk,
         512 free]; score matmuls run as row-tiled pairs on disjoint PE
         row groups (tile_position (0,0)/(64,0)) -> 2x score throughput.
  v    : bf16 token-major, per head-pair block of 192 [v_even(64)|mask(64)|v_odd(64)]
         The ctx matmul reads a 128-wide stationary window per head:
         even head -> rows 0:64 ctx + 64:128 denominator (mask block),
         odd head  -> rows 0:64 denominator + 64:128 ctx. The denominator
         arrives REPLICATED on 64 partitions, so softmax normalization is
         two full-width DVE ops (reciprocal + multiply) -- no partition
         broadcast, no single-lane ops.
  gel/qp/kp share one 24-slot [128,512]bf16 pool (disjoint lifetimes).

Schedule (single in-order PE stream, phases chosen so every serial
chain -- exp on ACT, LayerNorm on DVE, xbar retranspose -- hides under
independent matmuls):
  A: QK(s0)        B: V(s0)
  C: scores/ctx(s0) software-pipelined with QK(s1) chunks (exp latency
     covered by interleaved matmuls; numerators need only 6 live tiles)
  D: V(s1)
  E: scores/ctx(s1) pipelined with Wo(s0)+LN1(s0)+per-chunk retranspose
  F: Wo(s1)+LN1(s1)+retranspose
  G: FF1(s0)+gelu  H: FF2(s0)+LN2(s0)+retranspose (hidden under I/J)
  I: FF1(s1)       J: FF2(s1)+LN2(s1)+retranspose (hidden under next QK(s0))
LayerNorm rstd = exp(-0.5*ln(var+eps)) on ACT: ln+exp live in the same
activation-table set as softmax exp, so no Sqrt-table thrash.
All matmuls contract over the partition dim with fp32 PSUM accumulation.
Softmax skips max-subtraction (scores are O(1)).
"""

import os

import ml_dtypes
import numpy as np

B, L, D, NL, H, DK, FF = 16, 512, 768, 12, 12, 64, 3072
NCORES = 8
SPC = B // NCORES          # sequences per core
T = SPC * L                # tokens per core
DCH = D // 128             # 6 chunks of d
FCH = FF // 128            # 24 chunks of ff
TCH = T // 128             # 8 chunks of t
NP = H // 2                # 6 head pairs
PW = 3 * DK                # 192: per-pair v width [v_even | mask | v_odd]
VTW = NP * PW              # 1152

_CACHE = {}


def _build_program(nl):
    import concourse.mybir as mybir
    import concourse.tile as tile
    from concourse import bacc

    f32, bf16 = mybir.dt.float32, mybir.dt.bfloat16
    AF = mybir.ActivationFunctionType
    OP = mybir.AluOpType

    nc = bacc.Bacc("TRN2", target_bir_lowering=False, debug=False)
    h0_d = nc.dram_tensor("h0", [T, D], f32, kind="ExternalInput").ap()
    h0T_d = nc.dram_tensor("h0T", [SPC, D, L], bf16, kind="ExternalInput").ap()
    maskf_d = nc.dram_tensor("maskf", [T, 1], f32, kind="ExternalInput").ap()
    wq_d = nc.dram_tensor("wq", [nl, D, D], bf16, kind="ExternalInput").ap()
    wk_d = nc.dram_tensor("wk", [nl, D, D], bf16, kind="ExternalInput").ap()
    wv_d = nc.dram_tensor("wv", [nl, D, D], bf16, kind="ExternalInput").ap()
    wo_d = nc.dram_tensor("wo", [nl, D, D], bf16, kind="ExternalInput").ap()
    w1_d = nc.dram_tensor("w1", [nl, D, FF], bf16, kind="ExternalInput").ap()
    w2_d = nc.dram_tensor("w2", [nl, FF, D], bf16, kind="ExternalInput").ap()
    names = ["h0", "h0T", "maskf", "wq", "wk", "wv", "wo", "w1", "w2"]
    out_d = nc.dram_tensor("out", [T, D], f32, kind="ExternalOutput").ap()

    with tile.TileContext(nc) as tc:
        with (
            tc.tile_pool(name="const", bufs=1) as cp,
            tc.tile_pool(name="persist", bufs=1) as pp,
            tc.tile_pool(name="wts", bufs=1) as wp,
            tc.tile_pool(name="work", bufs=2) as wk,
            tc.tile_pool(name="stage", bufs=2, space="DRAM") as dp,
            tc.tile_pool(name="psum", bufs=2, space="PSUM") as psp,
        ):
            eps_t = cp.tile([128, 1], f32)
            nc.vector.memset(eps_t[:], 1e-5)
            mtile = cp.tile([128, TCH], f32)
            nc.sync.dma_start(mtile[:], maskf_d.rearrange("(i p) o -> p (i o)", p=128))

            h_t = [pp.tile([128, D], f32, tag=f"h{i}", name=f"h{i}") for i in range(TCH)]
            hT_s = [pp.tile([128, DCH, L], bf16, tag=f"hT{s}", name=f"hT{s}")
                    for s in range(SPC)]
            v_t = [pp.tile([128, VTW], bf16, tag=f"v{i}", name=f"v{i}")
                   for i in range(TCH)]

            # qp/kp (phases A-F) and gel (phases G-J) share one 24-slot pool
            def qkg_tile():
                return wk.tile([128, L], bf16, tag="qkg", bufs=24, name="qkg")

            def qk_dma(l, mat_d, dc):
                wc = wk.tile([128, DCH, 128], bf16, tag="wqkcol", bufs=6,
                             name="wqkc")
                nc.sync.dma_start(
                    wc[:], mat_d[l][:, dc * 128:(dc + 1) * 128]
                    .rearrange("(c p) n -> p c n", p=128))
                return wc

            def w1_dma(l, f):
                w1c = wk.tile([128, DCH, 128], bf16, tag="w1col", bufs=6,
                              name="w1c")
                nc.sync.dma_start(
                    w1c[:], w1_d[l][:, f * 128:(f + 1) * 128]
                    .rearrange("(c p) n -> p c n", p=128))
                return w1c

            # one-time init: first wq column-tiles, transposed embeddings
            # (sync queue; scalar carries ONLY xbar transposes), residuals,
            # v mask blocks.
            qk_pre = [qk_dma(0, wq_d, dc) for dc in range(DCH)]
            for s in range(SPC):
                for c in range(DCH):
                    nc.sync.dma_start(hT_s[s][:, c, :],
                                      h0T_d[s, c * 128:(c + 1) * 128, :])
            for i in range(TCH):
                nc.gpsimd.dma_start(h_t[i][:], h0_d[i * 128:(i + 1) * 128, :])
                vv = v_t[i][:].rearrange("p (pr w) -> p pr w", w=PW)
                nc.vector.memset(vv[:, :, DK:2 * DK], 1.0)
                nc.vector.tensor_scalar(vv[:, :, DK:2 * DK], vv[:, :, DK:2 * DK],
                                        mtile[:, i:i + 1], None, op0=OP.mult)

            def ln_stats(i, gmv, j):
                """bn stats for residual-added h_t[i] -> gmv[:, j, :]."""
                st = wk.tile([128, 2, 6], f32, tag="bnst", bufs=4, name="bnst")
                for g in range(2):
                    nc.vector.bn_stats(st[:, g, :], h_t[i][:, g * 384:(g + 1) * 384])
                nc.vector.bn_aggr(gmv[:, j, :], st[:])

            def ln_finish(s, gmv, last):
                """one batched Sqrt for the 4-tile group, then apply per tile
                (+ bf16 cast + per-chunk DRAM-staged xbar retranspose)."""
                gstd = wk.tile([128, 4], f32, tag="gstd", bufs=2, name="gstd")
                nc.scalar.activation(gstd[:], gmv[:, :, 1], AF.Sqrt, bias=eps_t[:])
                nc.vector.reciprocal_approx_fast(gstd[:], gstd[:])
                hst = None
                if not last:
                    hst = dp.tile([L, D], bf16, tag=f"hst{s}", name="hst")
                for j in range(4):
                    i = s * 4 + j
                    nc.vector.tensor_scalar(h_t[i][:], h_t[i][:], gmv[:, j, 0:1],
                                            gstd[:, j:j + 1],
                                            op0=OP.subtract, op1=OP.mult)
                    if not last:
                        hbt = wk.tile([128, D], bf16, tag="hb", bufs=4, name="hb")
                        nc.vector.tensor_copy(hbt[:], h_t[i][:])
                        nc.sync.dma_start(
                            hst[j * 128:(j + 1) * 128, :], hbt[:])
                        nc.scalar.dma_start_transpose(
                            hT_s[s][:, :, j * 128:(j + 1) * 128],
                            hst[j * 128:(j + 1) * 128, :])

            def gmv_tile():
                return wk.tile([128, 4, 2], f32, tag="gmv", bufs=2, name="gmv")

            def qk_chunk(s, wc, dstp, dc):
                ps = psp.tile([128, L], f32, tag="p5", bufs=4, name="psqk")
                for c in range(DCH):
                    nc.tensor.matmul(ps[:], wc[:, c, :], hT_s[s][:, c, :],
                                     start=(c == 0), stop=(c == DCH - 1))
                if s == 0:
                    nc.scalar.copy(dstp[dc][:], ps[:])
                else:
                    nc.vector.tensor_copy(dstp[dc][:], ps[:])

            def v_tile(s, j, wv_rows):
                i = s * 4 + j
                psA = psp.tile([128, L], f32, tag="p5", bufs=4, name="psA")
                psB = psp.tile([128, 256], f32, tag="p5", bufs=4, name="psB")
                for c in range(DCH):
                    stat = hT_s[s][:, c, j * 128:(j + 1) * 128]
                    nc.tensor.matmul(psA[:], stat, wv_rows[c][:, 0:512],
                                     start=(c == 0), stop=(c == DCH - 1))
                    nc.tensor.matmul(psB[:], stat, wv_rows[c][:, 512:768],
                                     start=(c == 0), stop=(c == DCH - 1))
                vv = v_t[i][:].rearrange("p (pr w) -> p pr w", w=PW)
                pav = psA[:].rearrange("p (pr w) -> p pr w", w=128)
                pbv = psB[:].rearrange("p (pr w) -> p pr w", w=128)
                m = mtile[:, i:i + 1]
                nc.vector.tensor_scalar(vv[:, 0:4, 0:DK], pav[:, :, 0:DK],
                                        m, None, op0=OP.mult)
                nc.vector.tensor_scalar(vv[:, 0:4, 2 * DK:PW], pav[:, :, DK:128],
                                        m, None, op0=OP.mult)
                nc.vector.tensor_scalar(vv[:, 4:6, 0:DK], pbv[:, :, 0:DK],
                                        m, None, op0=OP.mult)
                nc.vector.tensor_scalar(vv[:, 4:6, 2 * DK:PW], pbv[:, :, DK:128],
                                        m, None, op0=OP.mult)

            def sc_block(s, p, qp, kp):
                """scores + exp for head pair p; returns nm[hh][half] tiles."""
                nm = [[None, None], [None, None]]
                for half in range(2):
                    sc = [psp.tile([128, T], f32, tag="sc", bufs=2, name="sct")
                          for _ in range(2)]
                    for q in range(2):
                        tk = 2 * half + q
                        for hh in range(2):
                            nc.tensor.matmul(
                                sc[hh][:, q * L:(q + 1) * L],
                                kp[p][hh * DK:(hh + 1) * DK,
                                      tk * 128:(tk + 1) * 128],
                                qp[p][hh * DK:(hh + 1) * DK, :],
                                start=True, stop=True)
                    for hh in range(2):
                        t_nm = wk.tile([128, T], bf16, tag="numer", bufs=10,
                                       name="nm")
                        nc.scalar.activation(t_nm[:], sc[hh][:], AF.Exp)
                        nm[hh][half] = t_nm
                return nm

            def ctx_block(s, p, nm):
                """ctx + denominator-normalized write into hT_s[s]."""
                base = p * PW
                for hh in range(2):
                    cps = psp.tile([128, L], f32, tag="p5", bufs=4, name="cps")
                    for tk in range(4):
                        nc.tensor.matmul(
                            cps[:], v_t[s * 4 + tk][:, base + hh * DK:
                                                    base + hh * DK + 128],
                            nm[hh][tk // 2][:, (tk % 2) * L:(tk % 2 + 1) * L],
                            start=(tk == 0), stop=(tk == 3))
                    rec = wk.tile([DK, L], f32, tag="rec64", bufs=2, name="rec")
                    if hh == 0:
                        nc.vector.tensor_copy(rec[:], cps[DK:128, :])
                        nc.vector.reciprocal_approx_fast(rec[:], rec[:])
                        nc.vector.tensor_tensor(hT_s[s][0:DK, p, :],
                                                cps[0:DK, :], rec[:], op=OP.mult)
                    else:
                        nc.vector.tensor_copy(rec[:], cps[0:DK, :])
                        nc.vector.reciprocal_approx_fast(rec[:], rec[:])
                        nc.vector.tensor_tensor(hT_s[s][DK:128, p, :],
                                                cps[DK:128, :], rec[:], op=OP.mult)

            def wo_tile(s, j, wo_rows, gmv):
                i = s * 4 + j
                psA = psp.tile([128, L], f32, tag="p5", bufs=4, name="psA")
                psB = psp.tile([128, 256], f32, tag="p5", bufs=4, name="psB")
                for c in range(DCH):
                    stat = hT_s[s][:, c, j * 128:(j + 1) * 128]
                    nc.tensor.matmul(psA[:], stat, wo_rows[c][:, 0:512],
                                     start=(c == 0), stop=(c == DCH - 1))
                    nc.tensor.matmul(psB[:], stat, wo_rows[c][:, 512:768],
                                     start=(c == 0), stop=(c == DCH - 1))
                nc.vector.tensor_tensor(h_t[i][:, 0:512], psA[:],
                                        h_t[i][:, 0:512], op=OP.add)
                nc.vector.tensor_tensor(h_t[i][:, 512:768], psB[:],
                                        h_t[i][:, 512:768], op=OP.add)
                ln_stats(i, gmv, j)

            for l in range(nl):
                # bulk weight rows for this layer on the gpsimd (SWDGE) queue
                wv_rows, wo_rows, w2_rows = [], [], []
                for c in range(DCH):
                    wr = wp.tile([128, D], bf16, tag=f"wv{c}", name=f"wv{c}")
                    nc.gpsimd.dma_start(wr[:], wv_d[l, c * 128:(c + 1) * 128, :])
                    wv_rows.append(wr)
                for c in range(DCH):
                    wr = wp.tile([128, D], bf16, tag=f"wo{c}", name=f"wo{c}")
                    nc.gpsimd.dma_start(wr[:], wo_d[l, c * 128:(c + 1) * 128, :])
                    wo_rows.append(wr)
                for f in range(FCH):
                    wr = wp.tile([128, D], bf16, tag=f"w2r{f}", name=f"w2r{f}")
                    nc.gpsimd.dma_start(wr[:], w2_d[l, f * 128:(f + 1) * 128, :])
                    w2_rows.append(wr)

                qp = [[qkg_tile() for _ in range(NP)] for _ in range(SPC)]
                kp = [[qkg_tile() for _ in range(NP)] for _ in range(SPC)]

                # ---- A: QK(s0); K column-tiles stream behind the Q matmuls
                for dc in range(DCH):
                    qk_chunk(0, qk_pre[dc], qp[0], dc)
                kcols = [qk_dma(l, wk_d, dc) for dc in range(3)]
                for dc in range(DCH):
                    if dc + 3 < DCH:
                        kcols.append(qk_dma(l, wk_d, dc + 3))
                    qk_chunk(0, kcols[dc], kp[0], dc)

                # ---- B: V(s0)
                for j in range(4):
                    v_tile(0, j, wv_rows)

                # ---- C: attn(s0) pipelined with QK(s1) chunk stream
                s1q = ([("q", dc) for dc in range(DCH)]
                       + [("k", dc) for dc in range(DCH)])
                s1_loads = []
                for k in range(3):
                    m, dc = s1q[k]
                    s1_loads.append(qk_dma(l, wq_d if m == "q" else wk_d, dc))
                s1i = 0

                def s1_chunk():
                    nonlocal s1i
                    if s1i >= len(s1q):
                        return
                    if s1i + 3 < len(s1q):
                        m, dc = s1q[s1i + 3]
                        s1_loads.append(qk_dma(l, wq_d if m == "q" else wk_d, dc))
                    m, dc = s1q[s1i]
                    qk_chunk(1, s1_loads[s1i], qp[1] if m == "q" else kp[1], dc)
                    s1i += 1

                # pipeline: SC(p) ... CTX(p) two steps later, QK chunks filling
                nm_q = []
                for p in range(NP):
                    nm_q.append(sc_block(0, p, qp[0], kp[0]))
                    s1_chunk()
                    if p >= 1:
                        ctx_block(0, p - 1, nm_q[p - 1])
                        nm_q[p - 1] = None
                    s1_chunk()
                s1_chunk(); s1_chunk()
                ctx_block(0, NP - 1, nm_q[NP - 1])
                while s1i < len(s1q):
                    s1_chunk()

                # ---- D: V(s1)
                for j in range(4):
                    v_tile(1, j, wv_rows)

                # ---- E: attn(s1) pipelined with Wo(s0); LN1(s0) batch at end
                gmv0 = gmv_tile()
                nm_q = []
                wo_i = 0
                for p in range(NP):
                    nm_q.append(sc_block(1, p, qp[1], kp[1]))
                    if wo_i < 4:
                        wo_tile(0, wo_i, wo_rows, gmv0)
                        wo_i += 1
                    if p >= 1:
                        ctx_block(1, p - 1, nm_q[p - 1])
                        nm_q[p - 1] = None
                while wo_i < 4:
                    wo_tile(0, wo_i, wo_rows, gmv0)
                    wo_i += 1
                ln_finish(0, gmv0, last=False)
                ctx_block(1, NP - 1, nm_q[NP - 1])

                # ---- F: Wo(s1); LN1(s1) batch at end
                gmv1 = gmv_tile()
                for j in range(4):
                    wo_tile(1, j, wo_rows, gmv1)
                ln_finish(1, gmv1, last=False)

                # ---- G..J: per-seq FF1 -> FF2 (+LN2 + retranspose)
                w1_pre = [w1_dma(l, f) for f in range(3)]
                for s in range(SPC):
                    gel = []
                    for f in range(FCH):
                        w1c = w1_pre[f] if (s == 0 and f < 3) else w1_dma(l, f)
                        ps = psp.tile([128, L], f32, tag="p5", bufs=4, name="psf1")
                        for c in range(DCH):
                            nc.tensor.matmul(ps[:], w1c[:, c, :],
                                             hT_s[s][:, c, :],
                                             start=(c == 0), stop=(c == DCH - 1))
                        g = qkg_tile()
                        # BERT_SIM_TANH: CoreSim lacks Gelu; sub Tanh for
                        # local dataflow validation only (numpy mirror does too)
                        gelu_af = (AF.Tanh if os.environ.get("BERT_SIM_TANH")
                                   == "1" else AF.Gelu)
                        nc.scalar.activation(g[:], ps[:], gelu_af)
                        gel.append(g)

                    gmv2 = gmv_tile()
                    for dh in range(2):
                        for j in range(4):
                            i = s * 4 + j
                            ps = psp.tile([128, 384], f32, tag="p5", bufs=4, name="psf2")
                            for f in range(FCH):
                                nc.tensor.matmul(
                                    ps[:], gel[f][:, j * 128:(j + 1) * 128],
                                    w2_rows[f][:, dh * 384:(dh + 1) * 384],
                                    start=(f == 0), stop=(f == FCH - 1))
                            nc.vector.tensor_tensor(
                                h_t[i][:, dh * 384:(dh + 1) * 384], ps[:],
                                h_t[i][:, dh * 384:(dh + 1) * 384], op=OP.add)
                            if dh == 1:
                                ln_stats(i, gmv2, j)
                        if s == 0 and dh == 1 and l + 1 < nl:
                            qk_pre = [qk_dma(l + 1, wq_d, dc)
                                      for dc in range(DCH)]
                    ln_finish(s, gmv2, last=(l == nl - 1))

            for i in range(TCH):
                eng = nc.sync if i % 2 == 0 else nc.gpsimd
                eng.dma_start(out_d[i * 128:(i + 1) * 128, :], h_t[i][:])

    nc.compile()
    return nc, names


def _get_program(nl):
    if nl not in _CACHE:
        _CACHE[nl] = _build_program(nl)
    return _CACHE[nl]


def kernel(**inputs):
    from concourse import bass_utils

    x = np.asarray(inputs["x"])
    tok = np.asarray(inputs["token_emb"], np.float32)
    pe = np.asarray(inputs["pe"], np.float32)
    to_bf = lambda a: np.asarray(a, np.float32).astype(ml_dtypes.bfloat16)

    h0 = tok[x] + pe[None]                                   # (B, L, D) f32
    maskf = (x > 0).astype(np.float32)                       # (B, L)

    nl = int(os.environ.get("BERT_NL", str(NL)))
    bias_arrs = [np.asarray(inputs[k], np.float32)[:nl]
                 for k in ("bq", "bk", "bv", "bo", "b1", "b2")]
    assert not any(np.any(a != 0.0) for a in bias_arrs), "bias unsupported"
    lng = np.asarray(inputs["ln_g"], np.float32)[:nl]
    lnb = np.asarray(inputs["ln_b"], np.float32)[:nl]
    assert not (np.any(lng != 1.0) or np.any(lnb != 0.0)), "affine unsupported"

    nc, names = _get_program(nl)

    shared = {
        "wq": to_bf(np.asarray(inputs["Wq"][:nl], np.float32) * 0.125),
        "wk": to_bf(inputs["Wk"][:nl]),
        "wv": to_bf(inputs["Wv"][:nl]), "wo": to_bf(inputs["Wo"][:nl]),
        "w1": to_bf(inputs["W1"][:nl]), "w2": to_bf(inputs["W2"][:nl]),
    }

    in_maps = []
    for c in range(NCORES):
        im = dict(shared)
        hc = h0[SPC * c:SPC * (c + 1)]                       # (SPC, L, D)
        im["h0"] = np.ascontiguousarray(hc.reshape(T, D), dtype=np.float32)
        im["h0T"] = np.ascontiguousarray(
            hc.transpose(0, 2, 1)).astype(ml_dtypes.bfloat16)
        im["maskf"] = np.ascontiguousarray(
            maskf[SPC * c:SPC * (c + 1)].reshape(T, 1), dtype=np.float32)
        in_maps.append(im)

    trace = os.environ.get("BERT_TRACE", "0") == "1"
    res = bass_utils.run_bass_kernel_spmd(
        nc, in_maps, core_ids=list(range(NCORES)), trace=trace)
    if trace:
        print(f"HW exec time: {res.exec_time_ns} ns")
        try:
            import pickle
            insts, tpath = res.instructions_and_trace
            rows = [(i.engine, i.name,
                     f"{getattr(i, 'source_line', '')}", i.timestamp,
                     i.duration) for i in insts]
            with open("/root/problem/work/insts.pkl", "wb") as f:
                pickle.dump({"rows": rows, "trace_path": tpath,
                             "exec_time_ns": res.exec_time_ns}, f)
            print(f"trace dumped: {len(rows)} insts, {tpath}")
        except Exception as e:
            print("trace dump failed:", e)

    out = np.stack([np.asarray(res.results[c]["out"]).reshape(SPC, L, D)
                    for c in range(NCORES)])
    return out.reshape(B, L, D).astype(np.float32)


# revision 16
# speedup vs baseline: 1.2632x; 1.2632x over previous
"""BERT encoder forward pass on 8 TRN2 NeuronCores.

Strategy: pure data parallelism over the batch (16 sequences -> 2 per core).
Each core runs the full 12-layer encoder on its 2 sequences; no collectives.

Per-core layout (T = 2*512 = 1024 tokens, L = 512 per sequence):
  h_t  : residual accumulator, fp32, token-major      [t(8x128 part), d(768)]
  hT_s : per-sequence transposed activations, bf16    [d(128 part), dchunk(6), t(512)]
         refreshed per layer by whole-sequence xbar DMA transposes through
         a DRAM staging buffer (staging writes on the gpsimd SWDGE queue,
         transposes alone on the scalar HWDGE queue -> no xbar-mode flips).
         Attention context overwrites hT_s in place (WAR-tracked).
  qp/kp: q^T / k^T per (seq, head-pair), bf16         [128 part = 2 heads x 64dk,
         512 free]; score matmuls run as row-tiled pairs on disjoint PE
         row groups (tile_position (0,0)/(64,0)) -> 2x score throughput.
  v    : bf16 token-major, per head-pair block of 192 [v_even(64)|mask(64)|v_odd(64)]
         The ctx matmul reads a 128-wide stationary window per head:
         even head -> rows 0:64 ctx + 64:128 denominator (mask block),
         odd head  -> rows 0:64 denominator + 64:128 ctx. The denominator
         arrives REPLICATED on 64 partitions, so softmax normalization is
         three full-width DVE ops (copy + reciprocal + multiply) -- no
         gpsimd partition broadcast, no single-lane ops.
  gel/qp/kp share one 24-slot [128,512]bf16 pool (disjoint lifetimes).

Schedule (single in-order PE stream, phases chosen so every serial
chain -- exp on ACT, LayerNorm on DVE, xbar retranspose -- hides under
independent matmuls):
  A: QK(s0)        B: V(s0)
  C: scores/ctx(s0) software-pipelined with QK(s1) chunks (exp latency
     covered by interleaved matmuls; numerators need only ~6 live tiles)
  D: V(s1)
  E: scores/ctx(s1) pipelined with Wo(s0); LN1(s0) batch + retranspose at end
  F: Wo(s1); LN1(s1) batch + retranspose
  G: FF1(s0)+gelu  H: FF2(s0)+LN2(s0)+retranspose (hidden under I/J)
  I: FF1(s1)       J: FF2(s1)+LN2(s1)+retranspose (hidden under next QK(s0))
LayerNorm uses one batched Sqrt per 4-tile group (few ACT-table switches,
all placed between, not inside, the exp/gelu bursts).
All matmuls contract over the partition dim with fp32 PSUM accumulation.
Softmax skips max-subtraction (scores are O(1)).
"""

import os

import ml_dtypes
import numpy as np

B, L, D, NL, H, DK, FF = 16, 512, 768, 12, 12, 64, 3072
NCORES = 8
SPC = B // NCORES          # sequences per core
T = SPC * L                # tokens per core
DCH = D // 128             # 6 chunks of d
FCH = FF // 128            # 24 chunks of ff
TCH = T // 128             # 8 chunks of t
NP = H // 2                # 6 head pairs
PW = 3 * DK                # 192: per-pair v width [v_even | mask | v_odd]
VTW = NP * PW              # 1152

_CACHE = {}


def _build_program(nl):
    import concourse.mybir as mybir
    import concourse.tile as tile
    from concourse import bacc

    f32, bf16 = mybir.dt.float32, mybir.dt.bfloat16
    AF = mybir.ActivationFunctionType
    OP = mybir.AluOpType

    nc = bacc.Bacc("TRN2", target_bir_lowering=False, debug=False)
    h0_d = nc.dram_tensor("h0", [T, D], f32, kind="ExternalInput").ap()
    h0T_d = nc.dram_tensor("h0T", [SPC, D, L], bf16, kind="ExternalInput").ap()
    maskf_d = nc.dram_tensor("maskf", [T, 1], f32, kind="ExternalInput").ap()
    wq_d = nc.dram_tensor("wq", [nl, D, D], bf16, kind="ExternalInput").ap()
    wk_d = nc.dram_tensor("wk", [nl, D, D], bf16, kind="ExternalInput").ap()
    wv_d = nc.dram_tensor("wv", [nl, D, D], bf16, kind="ExternalInput").ap()
    wo_d = nc.dram_tensor("wo", [nl, D, D], bf16, kind="ExternalInput").ap()
    w1_d = nc.dram_tensor("w1", [nl, D, FF], bf16, kind="ExternalInput").ap()
    w2_d = nc.dram_tensor("w2", [nl, FF, D], bf16, kind="ExternalInput").ap()
    names = ["h0", "h0T", "maskf", "wq", "wk", "wv", "wo", "w1", "w2"]
    out_d = nc.dram_tensor("out", [T, D], f32, kind="ExternalOutput").ap()

    with tile.TileContext(nc) as tc:
        with (
            tc.tile_pool(name="const", bufs=1) as cp,
            tc.tile_pool(name="persist", bufs=1) as pp,
            tc.tile_pool(name="wts", bufs=1) as wp,
            tc.tile_pool(name="work", bufs=2) as wk,
            tc.tile_pool(name="stage", bufs=2, space="DRAM") as dp,
            tc.tile_pool(name="psum", bufs=2, space="PSUM") as psp,
        ):
            eps_t = cp.tile([128, 1], f32)
            nc.vector.memset(eps_t[:], 1e-5)
            mtile = cp.tile([128, TCH], f32)
            nc.sync.dma_start(mtile[:], maskf_d.rearrange("(i p) o -> p (i o)", p=128))

            h_t = [pp.tile([128, D], f32, tag=f"h{i}", name=f"h{i}") for i in range(TCH)]
            hT_s = [pp.tile([128, DCH, L], bf16, tag=f"hT{s}", name=f"hT{s}")
                    for s in range(SPC)]
            v_t = [pp.tile([128, VTW], bf16, tag=f"v{i}", name=f"v{i}")
                   for i in range(TCH)]

            # qp/kp (phases A-F) and gel (phases G-J) share one 24-slot pool
            def qkg_tile():
                return wk.tile([128, L], bf16, tag="qkg", bufs=24, name="qkg")

            def qk_dma(l, mat_d, dc):
                wc = wk.tile([128, DCH, 128], bf16, tag="wqkcol", bufs=6,
                             name="wqkc")
                nc.sync.dma_start(
                    wc[:], mat_d[l][:, dc * 128:(dc + 1) * 128]
                    .rearrange("(c p) n -> p c n", p=128))
                return wc

            def w1_dma(l, f):
                w1c = wk.tile([128, DCH, 128], bf16, tag="w1col", bufs=6,
                              name="w1c")
                nc.sync.dma_start(
                    w1c[:], w1_d[l][:, f * 128:(f + 1) * 128]
                    .rearrange("(c p) n -> p c n", p=128))
                return w1c

            # one-time init: first wq column-tiles, transposed embeddings
            # (sync queue; scalar carries ONLY xbar transposes), residuals,
            # v mask blocks.
            qk_pre = [qk_dma(0, wq_d, dc) for dc in range(DCH)]
            for s in range(SPC):
                for c in range(DCH):
                    nc.sync.dma_start(hT_s[s][:, c, :],
                                      h0T_d[s, c * 128:(c + 1) * 128, :])
            for i in range(TCH):
                nc.gpsimd.dma_start(h_t[i][:], h0_d[i * 128:(i + 1) * 128, :])
                vv = v_t[i][:].rearrange("p (pr w) -> p pr w", w=PW)
                nc.vector.memset(vv[:, :, DK:2 * DK], 1.0)
                nc.vector.tensor_scalar(vv[:, :, DK:2 * DK], vv[:, :, DK:2 * DK],
                                        mtile[:, i:i + 1], None, op0=OP.mult)

            def ln_stats(i, gmv, j):
                """bn stats for residual-added h_t[i] -> gmv[:, j, :]."""
                st = wk.tile([128, 2, 6], f32, tag="bnst", bufs=4, name="bnst")
                for g in range(2):
                    nc.vector.bn_stats(st[:, g, :], h_t[i][:, g * 384:(g + 1) * 384])
                nc.vector.bn_aggr(gmv[:, j, :], st[:])

            def ln_finish(s, gmv, last):
                """one batched Sqrt for the 4-tile group, then apply per tile,
                bf16 cast, gpsimd staging write, one whole-seq xbar transpose."""
                gstd = wk.tile([128, 4], f32, tag="gstd", bufs=2, name="gstd")
                nc.scalar.activation(gstd[:], gmv[:, :, 1], AF.Sqrt, bias=eps_t[:])
                nc.vector.reciprocal_approx_fast(gstd[:], gstd[:])
                hst = None
                if not last:
                    hst = dp.tile([L, D], bf16, tag=f"hst{s}", name="hst")
                for j in range(4):
                    i = s * 4 + j
                    nc.vector.tensor_scalar(h_t[i][:], h_t[i][:], gmv[:, j, 0:1],
                                            gstd[:, j:j + 1],
                                            op0=OP.subtract, op1=OP.mult)
                    if not last:
                        hbt = wk.tile([128, D], bf16, tag="hb", bufs=4, name="hb")
                        nc.vector.tensor_copy(hbt[:], h_t[i][:])
                        nc.gpsimd.dma_start(
                            hst[j * 128:(j + 1) * 128, :], hbt[:])
                if not last:
                    nc.scalar.dma_start_transpose(hT_s[s][:], hst[:])

            def gmv_tile():
                return wk.tile([128, 4, 2], f32, tag="gmv", bufs=2, name="gmv")

            def qk_chunk(s, wc, dstp, dc):
                ps = psp.tile([128, L], f32, tag="p5", bufs=4, name="psqk")
                for c in range(DCH):
                    nc.tensor.matmul(ps[:], wc[:, c, :], hT_s[s][:, c, :],
                                     start=(c == 0), stop=(c == DCH - 1))
                if s == 0:
                    nc.scalar.copy(dstp[dc][:], ps[:])
                else:
                    nc.vector.tensor_copy(dstp[dc][:], ps[:])

            def v_tile(s, j, wv_rows):
                i = s * 4 + j
                psA = psp.tile([128, L], f32, tag="p5", bufs=4, name="psA")
                psB = psp.tile([128, 256], f32, tag="p5", bufs=4, name="psB")
                for c in range(DCH):
                    stat = hT_s[s][:, c, j * 128:(j + 1) * 128]
                    nc.tensor.matmul(psA[:], stat, wv_rows[c][:, 0:512],
                                     start=(c == 0), stop=(c == DCH - 1))
                    nc.tensor.matmul(psB[:], stat, wv_rows[c][:, 512:768],
                                     start=(c == 0), stop=(c == DCH - 1))
                vv = v_t[i][:].rearrange("p (pr w) -> p pr w", w=PW)
                pav = psA[:].rearrange("p (pr w) -> p pr w", w=128)
                pbv = psB[:].rearrange("p (pr w) -> p pr w", w=128)
                m = mtile[:, i:i + 1]
                nc.vector.tensor_scalar(vv[:, 0:4, 0:DK], pav[:, :, 0:DK],
                                        m, None, op0=OP.mult)
                nc.vector.tensor_scalar(vv[:, 0:4, 2 * DK:PW], pav[:, :, DK:128],
                                        m, None, op0=OP.mult)
                nc.vector.tensor_scalar(vv[:, 4:6, 0:DK], pbv[:, :, 0:DK],
                                        m, None, op0=OP.mult)
                nc.vector.tensor_scalar(vv[:, 4:6, 2 * DK:PW], pbv[:, :, DK:128],
                                        m, None, op0=OP.mult)

            def sc_block(s, p, qpl, kpl):
                """scores + exp for head pair p; returns nm[hh][half] tiles."""
                nm = [[None, None], [None, None]]
                for half in range(2):
                    sc = [psp.tile([128, T], f32, tag="sc", bufs=2, name="sct")
                          for _ in range(2)]
                    for q in range(2):
                        tk = 2 * half + q
                        for hh in range(2):
                            nc.tensor.matmul(
                                sc[hh][:, q * L:(q + 1) * L],
                                kpl[p][hh * DK:(hh + 1) * DK,
                                       tk * 128:(tk + 1) * 128],
                                qpl[p][hh * DK:(hh + 1) * DK, :],
                                start=True, stop=True)
                    for hh in range(2):
                        t_nm = wk.tile([128, T], bf16, tag="numer", bufs=10,
                                       name="nm")
                        nc.scalar.activation(t_nm[:], sc[hh][:], AF.Exp)
                        nm[hh][half] = t_nm
                return nm

            def ctx_block(s, p, nm):
                """ctx + denominator-normalized write into hT_s[s]."""
                base = p * PW
                for hh in range(2):
                    cps = psp.tile([128, L], f32, tag="p5", bufs=4, name="cps")
                    for tk in range(4):
                        nc.tensor.matmul(
                            cps[:], v_t[s * 4 + tk][:, base + hh * DK:
                                                    base + hh * DK + 128],
                            nm[hh][tk // 2][:, (tk % 2) * L:(tk % 2 + 1) * L],
                            start=(tk == 0), stop=(tk == 3))
                    rec = wk.tile([DK, L], f32, tag="rec64", bufs=2, name="rec")
                    if hh == 0:
                        nc.vector.tensor_copy(rec[:], cps[DK:128, :])
                        nc.vector.reciprocal_approx_fast(rec[:], rec[:])
                        nc.vector.tensor_tensor(hT_s[s][0:DK, p, :],
                                                cps[0:DK, :], rec[:], op=OP.mult)
                    else:
                        nc.vector.tensor_copy(rec[:], cps[0:DK, :])
                        nc.vector.reciprocal_approx_fast(rec[:], rec[:])
                        nc.vector.tensor_tensor(hT_s[s][DK:128, p, :],
                                                cps[DK:128, :], rec[:], op=OP.mult)

            def wo_tile(s, j, wo_rows, gmv):
                i = s * 4 + j
                psA = psp.tile([128, L], f32, tag="p5", bufs=4, name="psA")
                psB = psp.tile([128, 256], f32, tag="p5", bufs=4, name="psB")
                for c in range(DCH):
                    stat = hT_s[s][:, c, j * 128:(j + 1) * 128]
                    nc.tensor.matmul(psA[:], stat, wo_rows[c][:, 0:512],
                                     start=(c == 0), stop=(c == DCH - 1))
                    nc.tensor.matmul(psB[:], stat, wo_rows[c][:, 512:768],
                                     start=(c == 0), stop=(c == DCH - 1))
                nc.vector.tensor_tensor(h_t[i][:, 0:512], psA[:],
                                        h_t[i][:, 0:512], op=OP.add)
                nc.vector.tensor_tensor(h_t[i][:, 512:768], psB[:],
                                        h_t[i][:, 512:768], op=OP.add)
                ln_stats(i, gmv, j)

            for l in range(nl):
                # bulk weight rows for this layer on the gpsimd (SWDGE) queue
                wv_rows, wo_rows, w2_rows = [], [], []
                for c in range(DCH):
                    wr = wp.tile([128, D], bf16, tag=f"wv{c}", name=f"wv{c}")
                    nc.gpsimd.dma_start(wr[:], wv_d[l, c * 128:(c + 1) * 128, :])
                    wv_rows.append(wr)
                for c in range(DCH):
                    wr = wp.tile([128, D], bf16, tag=f"wo{c}", name=f"wo{c}")
                    nc.gpsimd.dma_start(wr[:], wo_d[l, c * 128:(c + 1) * 128, :])
                    wo_rows.append(wr)
                for f in range(FCH):
                    wr = wp.tile([128, D], bf16, tag=f"w2r{f}", name=f"w2r{f}")
                    nc.gpsimd.dma_start(wr[:], w2_d[l, f * 128:(f + 1) * 128, :])
                    w2_rows.append(wr)

                qp = [[qkg_tile() for _ in range(NP)] for _ in range(SPC)]
                kp = [[qkg_tile() for _ in range(NP)] for _ in range(SPC)]

                # ---- A: QK(s0); K column-tiles stream behind the Q matmuls
                for dc in range(DCH):
                    qk_chunk(0, qk_pre[dc], qp[0], dc)
                kcols = [qk_dma(l, wk_d, dc) for dc in range(3)]
                for dc in range(DCH):
                    if dc + 3 < DCH:
                        kcols.append(qk_dma(l, wk_d, dc + 3))
                    qk_chunk(0, kcols[dc], kp[0], dc)

                # ---- B: V(s0)
                for j in range(4):
                    v_tile(0, j, wv_rows)

                # ---- C: attn(s0) pipelined with QK(s1) chunk stream
                s1q = ([("q", dc) for dc in range(DCH)]
                       + [("k", dc) for dc in range(DCH)])
                s1_loads = []
                for k in range(3):
                    m, dc = s1q[k]
                    s1_loads.append(qk_dma(l, wq_d if m == "q" else wk_d, dc))
                s1i = 0

                def s1_chunk():
                    nonlocal s1i
                    if s1i >= len(s1q):
                        return
                    if s1i + 3 < len(s1q):
                        m, dc = s1q[s1i + 3]
                        s1_loads.append(qk_dma(l, wq_d if m == "q" else wk_d, dc))
                    m, dc = s1q[s1i]
                    qk_chunk(1, s1_loads[s1i], qp[1] if m == "q" else kp[1], dc)
                    s1i += 1

                # pipeline: SC(p) ... CTX(p) two steps later, QK chunks filling
                nm_q = []
                for p in range(NP):
                    nm_q.append(sc_block(0, p, qp[0], kp[0]))
                    s1_chunk()
                    if p >= 1:
                        ctx_block(0, p - 1, nm_q[p - 1])
                        nm_q[p - 1] = None
                    s1_chunk()
                s1_chunk()
                s1_chunk()
                ctx_block(0, NP - 1, nm_q[NP - 1])
                while s1i < len(s1q):
                    s1_chunk()

                # ---- D: V(s1)
                for j in range(4):
                    v_tile(1, j, wv_rows)

                # ---- E: attn(s1) pipelined with Wo(s0); LN1(s0) batch at end
                gmv0 = gmv_tile()
                nm_q = []
                wo_i = 0
                for p in range(NP):
                    nm_q.append(sc_block(1, p, qp[1], kp[1]))
                    if wo_i < 4:
                        wo_tile(0, wo_i, wo_rows, gmv0)
                        wo_i += 1
                    if p >= 1:
                        ctx_block(1, p - 1, nm_q[p - 1])
                        nm_q[p - 1] = None
                while wo_i < 4:
                    wo_tile(0, wo_i, wo_rows, gmv0)
                    wo_i += 1
                ln_finish(0, gmv0, last=False)
                ctx_block(1, NP - 1, nm_q[NP - 1])

                # ---- F: Wo(s1); LN1(s1) batch at end
                gmv1 = gmv_tile()
                for j in range(4):
                    wo_tile(1, j, wo_rows, gmv1)
                ln_finish(1, gmv1, last=False)

                # ---- G..J: per-seq FF1 -> FF2 (+LN2 + retranspose)
                w1_pre = [w1_dma(l, f) for f in range(3)]
                for s in range(SPC):
                    gel = []
                    for f in range(FCH):
                        w1c = w1_pre[f] if (s == 0 and f < 3) else w1_dma(l, f)
                        ps = psp.tile([128, L], f32, tag="p5", bufs=4, name="psf1")
                        for c in range(DCH):
                            nc.tensor.matmul(ps[:], w1c[:, c, :],
                                             hT_s[s][:, c, :],
                                             start=(c == 0), stop=(c == DCH - 1))
                        g = qkg_tile()
                        # BERT_SIM_TANH: CoreSim lacks Gelu; sub Tanh for
                        # local dataflow validation only (numpy mirror too)
                        gelu_af = (AF.Tanh if os.environ.get("BERT_SIM_TANH")
                                   == "1" else AF.Gelu)
                        nc.scalar.activation(g[:], ps[:], gelu_af)
                        gel.append(g)

                    gmv2 = gmv_tile()
                    for dh in range(2):
                        for j in range(4):
                            i = s * 4 + j
                            ps = psp.tile([128, 384], f32, tag="p5", bufs=4,
                                          name="psf2")
                            for f in range(FCH):
                                nc.tensor.matmul(
                                    ps[:], gel[f][:, j * 128:(j + 1) * 128],
                                    w2_rows[f][:, dh * 384:(dh + 1) * 384],
                                    start=(f == 0), stop=(f == FCH - 1))
                            nc.vector.tensor_tensor(
                                h_t[i][:, dh * 384:(dh + 1) * 384], ps[:],
                                h_t[i][:, dh * 384:(dh + 1) * 384], op=OP.add)
                            if dh == 1:
                                ln_stats(i, gmv2, j)
                        if s == 0 and dh == 1 and l + 1 < nl:
                            qk_pre = [qk_dma(l + 1, wq_d, dc)
                                      for dc in range(DCH)]
                    ln_finish(s, gmv2, last=(l == nl - 1))

            for i in range(TCH):
                eng = nc.sync if i % 2 == 0 else nc.gpsimd
                eng.dma_start(out_d[i * 128:(i + 1) * 128, :], h_t[i][:])

    nc.compile()
    return nc, names


def _get_program(nl):
    if nl not in _CACHE:
        _CACHE[nl] = _build_program(nl)
    return _CACHE[nl]


def kernel(**inputs):
    from concourse import bass_utils

    x = np.asarray(inputs["x"])
    tok = np.asarray(inputs["token_emb"], np.float32)
    pe = np.asarray(inputs["pe"], np.float32)
    to_bf = lambda a: np.asarray(a, np.float32).astype(ml_dtypes.bfloat16)

    h0 = tok[x] + pe[None]                                   # (B, L, D) f32
    maskf = (x > 0).astype(np.float32)                       # (B, L)

    nl = int(os.environ.get("BERT_NL", str(NL)))
    bias_arrs = [np.asarray(inputs[k], np.float32)[:nl]
                 for k in ("bq", "bk", "bv", "bo", "b1", "b2")]
    assert not any(np.any(a != 0.0) for a in bias_arrs), "bias unsupported"
    lng = np.asarray(inputs["ln_g"], np.float32)[:nl]
    lnb = np.asarray(inputs["ln_b"], np.float32)[:nl]
    assert not (np.any(lng != 1.0) or np.any(lnb != 0.0)), "affine unsupported"

    nc, names = _get_program(nl)

    shared = {
        "wq": to_bf(np.asarray(inputs["Wq"][:nl], np.float32) * 0.125),
        "wk": to_bf(inputs["Wk"][:nl]),
        "wv": to_bf(inputs["Wv"][:nl]), "wo": to_bf(inputs["Wo"][:nl]),
        "w1": to_bf(inputs["W1"][:nl]), "w2": to_bf(inputs["W2"][:nl]),
    }

    in_maps = []
    for c in range(NCORES):
        im = dict(shared)
        hc = h0[SPC * c:SPC * (c + 1)]                       # (SPC, L, D)
        im["h0"] = np.ascontiguousarray(hc.reshape(T, D), dtype=np.float32)
        im["h0T"] = np.ascontiguousarray(
            hc.transpose(0, 2, 1)).astype(ml_dtypes.bfloat16)
        im["maskf"] = np.ascontiguousarray(
            maskf[SPC * c:SPC * (c + 1)].reshape(T, 1), dtype=np.float32)
        in_maps.append(im)

    trace = os.environ.get("BERT_TRACE", "0") == "1"
    res = bass_utils.run_bass_kernel_spmd(
        nc, in_maps, core_ids=list(range(NCORES)), trace=trace)
    if trace:
        print(f"HW exec time: {res.exec_time_ns} ns")
        try:
            import pickle
            insts, tpath = res.instructions_and_trace
            rows = [(i.engine, i.name,
                     f"{getattr(i, 'source_line', '')}", i.timestamp,
                     i.duration) for i in insts]
            with open("/root/problem/work/insts.pkl", "wb") as f:
                pickle.dump({"rows": rows, "trace_path": tpath,
                             "exec_time_ns": res.exec_time_ns}, f)
            print(f"trace dumped: {len(rows)} insts, {tpath}")
        except Exception as e:
            print("trace dump failed:", e)

    out = np.stack([np.asarray(res.results[c]["out"]).reshape(SPC, L, D)
                    for c in range(NCORES)])
    return out.reshape(B, L, D).astype(np.float32)


# revision 21
# speedup vs baseline: 1.2695x; 1.0050x over previous
"""BERT encoder forward pass on 8 TRN2 NeuronCores.

Strategy: pure data parallelism over the batch (16 sequences -> 2 per core).
Each core runs the full 12-layer encoder on its 2 sequences; no collectives.

Per-core layout (T = 2*512 = 1024 tokens, L = 512 per sequence):
  h_t  : residual accumulator, fp32, token-major      [t(8x128 part), d(768)]
  hT_s : per-sequence transposed activations, bf16    [d(128 part), dchunk(6), t(512)]
         refreshed per layer by whole-sequence xbar DMA transposes through
         a DRAM staging buffer (staging writes on the gpsimd SWDGE queue,
         transposes alone on the scalar HWDGE queue -> no xbar-mode flips).
         Attention context overwrites hT_s in place (WAR-tracked).
  qp/kp: q^T / k^T per (seq, head-pair), bf16         [128 part = 2 heads x 64dk,
         512 free]; score matmuls run as row-tiled pairs on disjoint PE
         row groups (tile_position (0,0)/(64,0)) -> 2x score throughput.
  v    : bf16 token-major, per head-pair block of 192 [v_even(64)|mask(64)|v_odd(64)]
         The ctx matmul reads a 128-wide stationary window per head:
         even head -> rows 0:64 ctx + 64:128 denominator (mask block),
         odd head  -> rows 0:64 denominator + 64:128 ctx. The denominator
         arrives REPLICATED on 64 partitions, so softmax normalization is
         three full-width DVE ops (copy + reciprocal + multiply) -- no
         gpsimd partition broadcast, no single-lane ops.
  gel/qp/kp share one 24-slot [128,512]bf16 pool (disjoint lifetimes).

Schedule (single in-order PE stream, phases chosen so every serial
chain -- exp on ACT, LayerNorm on DVE, xbar retranspose -- hides under
independent matmuls):
  A: QK(s0)        B: V(s0)
  C: scores/ctx(s0) software-pipelined with QK(s1) chunks (exp latency
     covered by interleaved matmuls; numerators need only ~6 live tiles)
  D: V(s1)
  E: scores/ctx(s1) pipelined with Wo(s0); LN1(s0) batch + retranspose at end
  F: Wo(s1); LN1(s1) batch + retranspose
  G: FF1(s0)+gelu  H: FF2(s0)+LN2(s0)+retranspose (hidden under I/J)
  I: FF1(s1)       J: FF2(s1)+LN2(s1)+retranspose (hidden under next QK(s0))
LayerNorm uses one batched Sqrt per 4-tile group (few ACT-table switches,
all placed between, not inside, the exp/gelu bursts).
All matmuls contract over the partition dim with fp32 PSUM accumulation.
Softmax skips max-subtraction (scores are O(1)).
"""

import os

import ml_dtypes
import numpy as np

B, L, D, NL, H, DK, FF = 16, 512, 768, 12, 12, 64, 3072
NCORES = 8
SPC = B // NCORES          # sequences per core
T = SPC * L                # tokens per core
DCH = D // 128             # 6 chunks of d
FCH = FF // 128            # 24 chunks of ff
TCH = T // 128             # 8 chunks of t
NP = H // 2                # 6 head pairs
PW = 3 * DK                # 192: per-pair v width [v_even | mask | v_odd]
VTW = NP * PW              # 1152

_CACHE = {}


def _build_program(nl):
    import concourse.mybir as mybir
    import concourse.tile as tile
    from concourse import bacc

    f32, bf16 = mybir.dt.float32, mybir.dt.bfloat16
    AF = mybir.ActivationFunctionType
    OP = mybir.AluOpType

    nc = bacc.Bacc("TRN2", target_bir_lowering=False, debug=False)
    h0_d = nc.dram_tensor("h0", [T, D], f32, kind="ExternalInput").ap()
    h0T_d = nc.dram_tensor("h0T", [SPC, D, L], bf16, kind="ExternalInput").ap()
    maskf_d = nc.dram_tensor("maskf", [T, 1], f32, kind="ExternalInput").ap()
    wq_d = nc.dram_tensor("wq", [nl, D, D], bf16, kind="ExternalInput").ap()
    wk_d = nc.dram_tensor("wk", [nl, D, D], bf16, kind="ExternalInput").ap()
    wv_d = nc.dram_tensor("wv", [nl, D, D], bf16, kind="ExternalInput").ap()
    wo_d = nc.dram_tensor("wo", [nl, D, D], bf16, kind="ExternalInput").ap()
    w1_d = nc.dram_tensor("w1", [nl, D, FF], bf16, kind="ExternalInput").ap()
    w2_d = nc.dram_tensor("w2", [nl, FF, D], bf16, kind="ExternalInput").ap()
    names = ["h0", "h0T", "maskf", "wq", "wk", "wv", "wo", "w1", "w2"]
    out_d = nc.dram_tensor("out", [T, D], f32, kind="ExternalOutput").ap()

    with tile.TileContext(nc) as tc:
        with (
            tc.tile_pool(name="const", bufs=1) as cp,
            tc.tile_pool(name="persist", bufs=1) as pp,
            tc.tile_pool(name="wts", bufs=1) as wp,
            tc.tile_pool(name="work", bufs=2) as wk,
            tc.tile_pool(name="stage", bufs=2, space="DRAM") as dp,
            tc.tile_pool(name="psum", bufs=2, space="PSUM") as psp,
        ):
            eps_t = cp.tile([128, 1], f32)
            nc.vector.memset(eps_t[:], 1e-5)
            mtile = cp.tile([128, TCH], f32)
            nc.sync.dma_start(mtile[:], maskf_d.rearrange("(i p) o -> p (i o)", p=128))

            h_t = [pp.tile([128, D], f32, tag=f"h{i}", name=f"h{i}") for i in range(TCH)]
            hT_s = [pp.tile([128, DCH, L], bf16, tag=f"hT{s}", name=f"hT{s}")
                    for s in range(SPC)]
            v_t = [pp.tile([128, VTW], bf16, tag=f"v{i}", name=f"v{i}")
                   for i in range(TCH)]

            # qp/kp (phases A-F) and gel (phases G-J) share one 24-slot pool
            def qkg_tile():
                return wk.tile([128, L], bf16, tag="qkg", bufs=24, name="qkg")

            def qk_dma(l, mat_d, dc):
                wc = wk.tile([128, DCH, 128], bf16, tag="wqkcol", bufs=6,
                             name="wqkc")
                nc.sync.dma_start(
                    wc[:], mat_d[l][:, dc * 128:(dc + 1) * 128]
                    .rearrange("(c p) n -> p c n", p=128))
                return wc

            def w1_dma(l, f):
                w1c = wk.tile([128, DCH, 128], bf16, tag="w1col", bufs=6,
                              name="w1c")
                nc.sync.dma_start(
                    w1c[:], w1_d[l][:, f * 128:(f + 1) * 128]
                    .rearrange("(c p) n -> p c n", p=128))
                return w1c

            # one-time init: first wq column-tiles, transposed embeddings
            # (sync queue; scalar carries ONLY xbar transposes), residuals,
            # v mask blocks.
            qk_pre = [qk_dma(0, wq_d, dc) for dc in range(DCH)]
            for s in range(SPC):
                for c in range(DCH):
                    nc.sync.dma_start(hT_s[s][:, c, :],
                                      h0T_d[s, c * 128:(c + 1) * 128, :])
            for i in range(TCH):
                nc.gpsimd.dma_start(h_t[i][:], h0_d[i * 128:(i + 1) * 128, :])
                vv = v_t[i][:].rearrange("p (pr w) -> p pr w", w=PW)
                nc.vector.memset(vv[:, :, DK:2 * DK], 1.0)
                nc.vector.tensor_scalar(vv[:, :, DK:2 * DK], vv[:, :, DK:2 * DK],
                                        mtile[:, i:i + 1], None, op0=OP.mult)

            def ln_stats(i, gmv, j):
                """bn stats for residual-added h_t[i] -> gmv[:, j, :]."""
                st = wk.tile([128, 2, 6], f32, tag="bnst", bufs=4, name="bnst")
                for g in range(2):
                    nc.vector.bn_stats(st[:, g, :], h_t[i][:, g * 384:(g + 1) * 384])
                nc.vector.bn_aggr(gmv[:, j, :], st[:])

            def ln_finish(s, gmv, last):
                """one batched Sqrt for the 4-tile group, then apply per tile,
                bf16 cast, gpsimd staging write, one whole-seq xbar transpose."""
                gstd = wk.tile([128, 4], f32, tag="gstd", bufs=2, name="gstd")
                nc.scalar.activation(gstd[:], gmv[:, :, 1], AF.Sqrt, bias=eps_t[:])
                nc.vector.reciprocal_approx_fast(gstd[:], gstd[:])
                hst = None
                if not last:
                    hst = dp.tile([L, D], bf16, tag=f"hst{s}", name="hst")
                for j in range(4):
                    i = s * 4 + j
                    nc.vector.tensor_scalar(h_t[i][:], h_t[i][:], gmv[:, j, 0:1],
                                            gstd[:, j:j + 1],
                                            op0=OP.subtract, op1=OP.mult)
                    if not last:
                        hbt = wk.tile([128, D], bf16, tag="hb", bufs=4, name="hb")
                        nc.vector.tensor_copy(hbt[:], h_t[i][:])
                        nc.gpsimd.dma_start(
                            hst[j * 128:(j + 1) * 128, :], hbt[:])
                if not last:
                    nc.scalar.dma_start_transpose(hT_s[s][:], hst[:])

            def gmv_tile():
                return wk.tile([128, 4, 2], f32, tag="gmv", bufs=2, name="gmv")

            def qk_chunk(s, wc, dstp, dc):
                ps = psp.tile([128, L], f32, tag="p5", bufs=4, name="psqk")
                for c in range(DCH):
                    nc.tensor.matmul(ps[:], wc[:, c, :], hT_s[s][:, c, :],
                                     start=(c == 0), stop=(c == DCH - 1))
                if s == 0:
                    nc.scalar.copy(dstp[dc][:], ps[:])
                else:
                    nc.vector.tensor_copy(dstp[dc][:], ps[:])

            def v_tile(s, j, wv_rows):
                i = s * 4 + j
                psA = psp.tile([128, L], f32, tag="p5", bufs=4, name="psA")
                psB = psp.tile([128, 256], f32, tag="p5", bufs=4, name="psB")
                for c in range(DCH):
                    stat = hT_s[s][:, c, j * 128:(j + 1) * 128]
                    nc.tensor.matmul(psA[:], stat, wv_rows[c][:, 0:512],
                                     start=(c == 0), stop=(c == DCH - 1))
                    nc.tensor.matmul(psB[:], stat, wv_rows[c][:, 512:768],
                                     start=(c == 0), stop=(c == DCH - 1))
                vv = v_t[i][:].rearrange("p (pr w) -> p pr w", w=PW)
                pav = psA[:].rearrange("p (pr w) -> p pr w", w=128)
                pbv = psB[:].rearrange("p (pr w) -> p pr w", w=128)
                m = mtile[:, i:i + 1]
                nc.vector.tensor_scalar(vv[:, 0:4, 0:DK], pav[:, :, 0:DK],
                                        m, None, op0=OP.mult)
                nc.vector.tensor_scalar(vv[:, 0:4, 2 * DK:PW], pav[:, :, DK:128],
                                        m, None, op0=OP.mult)
                nc.vector.tensor_scalar(vv[:, 4:6, 0:DK], pbv[:, :, 0:DK],
                                        m, None, op0=OP.mult)
                nc.vector.tensor_scalar(vv[:, 4:6, 2 * DK:PW], pbv[:, :, DK:128],
                                        m, None, op0=OP.mult)

            def sc_block(s, p, qpl, kpl):
                """scores + exp for head pair p; returns nm[hh][half] tiles."""
                nm = [[None, None], [None, None]]
                for half in range(2):
                    sc = [psp.tile([128, T], f32, tag="sc", bufs=2, name="sct")
                          for _ in range(2)]
                    for q in range(2):
                        tk = 2 * half + q
                        for hh in range(2):
                            nc.tensor.matmul(
                                sc[hh][:, q * L:(q + 1) * L],
                                kpl[p][hh * DK:(hh + 1) * DK,
                                       tk * 128:(tk + 1) * 128],
                                qpl[p][hh * DK:(hh + 1) * DK, :],
                                start=True, stop=True)
                    for hh in range(2):
                        t_nm = wk.tile([128, T], bf16, tag="numer", bufs=10,
                                       name="nm")
                        nc.scalar.activation(t_nm[:], sc[hh][:], AF.Exp)
                        nm[hh][half] = t_nm
                return nm

            def ctx_block(s, p, nm):
                """ctx + denominator-normalized write into hT_s[s]."""
                base = p * PW
                for hh in range(2):
                    cps = psp.tile([128, L], f32, tag="p5", bufs=4, name="cps")
                    for tk in range(4):
                        nc.tensor.matmul(
                            cps[:], v_t[s * 4 + tk][:, base + hh * DK:
                                                    base + hh * DK + 128],
                            nm[hh][tk // 2][:, (tk % 2) * L:(tk % 2 + 1) * L],
                            start=(tk == 0), stop=(tk == 3))
                    rec = wk.tile([DK, L], f32, tag="rec64", bufs=2, name="rec")
                    if hh == 0:
                        nc.vector.tensor_copy(rec[:], cps[DK:128, :])
                        nc.vector.reciprocal_approx_fast(rec[:], rec[:])
                        nc.vector.tensor_tensor(hT_s[s][0:DK, p, :],
                                                cps[0:DK, :], rec[:], op=OP.mult)
                    else:
                        nc.vector.tensor_copy(rec[:], cps[0:DK, :])
                        nc.vector.reciprocal_approx_fast(rec[:], rec[:])
                        nc.vector.tensor_tensor(hT_s[s][DK:128, p, :],
                                                cps[DK:128, :], rec[:], op=OP.mult)

            def wo_tile(s, j, wo_rows, gmv):
                i = s * 4 + j
                psA = psp.tile([128, L], f32, tag="p5", bufs=4, name="psA")
                psB = psp.tile([128, 256], f32, tag="p5", bufs=4, name="psB")
                for c in range(DCH):
                    stat = hT_s[s][:, c, j * 128:(j + 1) * 128]
                    nc.tensor.matmul(psA[:], stat, wo_rows[c][:, 0:512],
                                     start=(c == 0), stop=(c == DCH - 1))
                    nc.tensor.matmul(psB[:], stat, wo_rows[c][:, 512:768],
                                     start=(c == 0), stop=(c == DCH - 1))
                nc.vector.tensor_tensor(h_t[i][:, 0:512], psA[:],
                                        h_t[i][:, 0:512], op=OP.add)
                nc.vector.tensor_tensor(h_t[i][:, 512:768], psB[:],
                                        h_t[i][:, 512:768], op=OP.add)
                ln_stats(i, gmv, j)

            for l in range(nl):
                # bulk weight rows for this layer on the gpsimd (SWDGE) queue
                wv_rows, wo_rows, w2_rows = [], [], []
                for c in range(DCH):
                    wr = wp.tile([128, D], bf16, tag=f"wv{c}", name=f"wv{c}")
                    nc.gpsimd.dma_start(wr[:], wv_d[l, c * 128:(c + 1) * 128, :])
                    wv_rows.append(wr)
                for c in range(DCH):
                    wr = wp.tile([128, D], bf16, tag=f"wo{c}", name=f"wo{c}")
                    nc.gpsimd.dma_start(wr[:], wo_d[l, c * 128:(c + 1) * 128, :])
                    wo_rows.append(wr)
                for f in range(FCH):
                    wr = wp.tile([128, D], bf16, tag=f"w2r{f}", name=f"w2r{f}")
                    nc.gpsimd.dma_start(wr[:], w2_d[l, f * 128:(f + 1) * 128, :])
                    w2_rows.append(wr)

                qp = [[qkg_tile() for _ in range(NP)] for _ in range(SPC)]
                kp = [[qkg_tile() for _ in range(NP)] for _ in range(SPC)]

                # ---- A: QK(s0); K column-tiles stream behind the Q matmuls
                for dc in range(DCH):
                    qk_chunk(0, qk_pre[dc], qp[0], dc)
                kcols = [qk_dma(l, wk_d, dc) for dc in range(3)]
                for dc in range(DCH):
                    if dc + 3 < DCH:
                        kcols.append(qk_dma(l, wk_d, dc + 3))
                    qk_chunk(0, kcols[dc], kp[0], dc)

                # ---- B: V(s0)
                for j in range(4):
                    v_tile(0, j, wv_rows)

                # ---- C: attn(s0) pipelined with QK(s1) chunk stream
                s1q = ([("q", dc) for dc in range(DCH)]
                       + [("k", dc) for dc in range(DCH)])
                s1_loads = []
                for k in range(3):
                    m, dc = s1q[k]
                    s1_loads.append(qk_dma(l, wq_d if m == "q" else wk_d, dc))
                s1i = 0

                def s1_chunk():
                    nonlocal s1i
                    if s1i >= len(s1q):
                        return
                    if s1i + 3 < len(s1q):
                        m, dc = s1q[s1i + 3]
                        s1_loads.append(qk_dma(l, wq_d if m == "q" else wk_d, dc))
                    m, dc = s1q[s1i]
                    qk_chunk(1, s1_loads[s1i], qp[1] if m == "q" else kp[1], dc)
                    s1i += 1

                # pipeline: SC(p) ... CTX(p) two steps later, QK chunks filling
                nm_q = []
                for p in range(NP):
                    nm_q.append(sc_block(0, p, qp[0], kp[0]))
                    s1_chunk()
                    if p >= 1:
                        ctx_block(0, p - 1, nm_q[p - 1])
                        nm_q[p - 1] = None
                    s1_chunk()
                s1_chunk()
                s1_chunk()
                ctx_block(0, NP - 1, nm_q[NP - 1])
                while s1i < len(s1q):
                    s1_chunk()

                # ---- D: V(s1)
                for j in range(4):
                    v_tile(1, j, wv_rows)

                # ---- E: attn(s1) pipelined with Wo(s0); LN1(s0) batch at end
                gmv0 = gmv_tile()
                nm_q = []
                wo_i = 0
                for p in range(NP):
                    nm_q.append(sc_block(1, p, qp[1], kp[1]))
                    if wo_i < 4:
                        wo_tile(0, wo_i, wo_rows, gmv0)
                        wo_i += 1
                    if p == 4:
                        # LN1(s0) batch as soon as the 4th tile's stats exist:
                        # the sqrt->apply->stage->transpose chain hides under
                        # the remaining scores/ctx + Wo(s1) matmuls
                        ln_finish(0, gmv0, last=False)
                    if p >= 1:
                        ctx_block(1, p - 1, nm_q[p - 1])
                        nm_q[p - 1] = None
                ctx_block(1, NP - 1, nm_q[NP - 1])

                # ---- F: Wo(s1); LN1(s1) batch at end
                gmv1 = gmv_tile()
                for j in range(4):
                    wo_tile(1, j, wo_rows, gmv1)
                ln_finish(1, gmv1, last=False)

                # ---- G..J: per-seq FF1 -> FF2 (+LN2 + retranspose)
                w1_pre = [w1_dma(l, f) for f in range(3)]
                for s in range(SPC):
                    gel = []
                    for f in range(FCH):
                        w1c = w1_pre[f] if f < 3 else w1_dma(l, f)
                        ps = psp.tile([128, L], f32, tag="p5", bufs=4, name="psf1")
                        for c in range(DCH):
                            nc.tensor.matmul(ps[:], w1c[:, c, :],
                                             hT_s[s][:, c, :],
                                             start=(c == 0), stop=(c == DCH - 1))
                        g = qkg_tile()
                        # BERT_SIM_TANH: CoreSim lacks Gelu; sub Tanh for
                        # local dataflow validation only (numpy mirror too)
                        gelu_af = (AF.Tanh if os.environ.get("BERT_SIM_TANH")
                                   == "1" else AF.Gelu)
                        nc.scalar.activation(g[:], ps[:], gelu_af)
                        gel.append(g)

                    gmv2 = gmv_tile()
                    for dh in range(2):
                        for j in range(4):
                            i = s * 4 + j
                            ps = psp.tile([128, 384], f32, tag="p5", bufs=4,
                                          name="psf2")
                            for f in range(FCH):
                                nc.tensor.matmul(
                                    ps[:], gel[f][:, j * 128:(j + 1) * 128],
                                    w2_rows[f][:, dh * 384:(dh + 1) * 384],
                                    start=(f == 0), stop=(f == FCH - 1))
                            nc.vector.tensor_tensor(
                                h_t[i][:, dh * 384:(dh + 1) * 384], ps[:],
                                h_t[i][:, dh * 384:(dh + 1) * 384], op=OP.add)
                            if dh == 1:
                                ln_stats(i, gmv2, j)
                        if s == 0 and dh == 0:
                            # prefetch seq-1's first FF1 column tiles
                            w1_pre = [w1_dma(l, f) for f in range(3)]
                        if s == 0 and dh == 1 and l + 1 < nl:
                            qk_pre = [qk_dma(l + 1, wq_d, dc)
                                      for dc in range(DCH)]
                    ln_finish(s, gmv2, last=(l == nl - 1))

            for i in range(TCH):
                eng = nc.sync if i % 2 == 0 else nc.gpsimd
                eng.dma_start(out_d[i * 128:(i + 1) * 128, :], h_t[i][:])

    nc.compile()
    return nc, names


def _get_program(nl):
    if nl not in _CACHE:
        _CACHE[nl] = _build_program(nl)
    return _CACHE[nl]


def kernel(**inputs):
    from concourse import bass_utils

    x = np.asarray(inputs["x"])
    tok = np.asarray(inputs["token_emb"], np.float32)
    pe = np.asarray(inputs["pe"], np.float32)
    to_bf = lambda a: np.asarray(a, np.float32).astype(ml_dtypes.bfloat16)

    h0 = tok[x] + pe[None]                                   # (B, L, D) f32
    maskf = (x > 0).astype(np.float32)                       # (B, L)

    nl = int(os.environ.get("BERT_NL", str(NL)))
    bias_arrs = [np.asarray(inputs[k], np.float32)[:nl]
                 for k in ("bq", "bk", "bv", "bo", "b1", "b2")]
    assert not any(np.any(a != 0.0) for a in bias_arrs), "bias unsupported"
    lng = np.asarray(inputs["ln_g"], np.float32)[:nl]
    lnb = np.asarray(inputs["ln_b"], np.float32)[:nl]
    assert not (np.any(lng != 1.0) or np.any(lnb != 0.0)), "affine unsupported"

    nc, names = _get_program(nl)

    shared = {
        "wq": to_bf(np.asarray(inputs["Wq"][:nl], np.float32) * 0.125),
        "wk": to_bf(inputs["Wk"][:nl]),
        "wv": to_bf(inputs["Wv"][:nl]), "wo": to_bf(inputs["Wo"][:nl]),
        "w1": to_bf(inputs["W1"][:nl]), "w2": to_bf(inputs["W2"][:nl]),
    }

    in_maps = []
    for c in range(NCORES):
        im = dict(shared)
        hc = h0[SPC * c:SPC * (c + 1)]                       # (SPC, L, D)
        im["h0"] = np.ascontiguousarray(hc.reshape(T, D), dtype=np.float32)
        im["h0T"] = np.ascontiguousarray(
            hc.transpose(0, 2, 1)).astype(ml_dtypes.bfloat16)
        im["maskf"] = np.ascontiguousarray(
            maskf[SPC * c:SPC * (c + 1)].reshape(T, 1), dtype=np.float32)
        in_maps.append(im)

    trace = os.environ.get("BERT_TRACE", "0") == "1"
    res = bass_utils.run_bass_kernel_spmd(
        nc, in_maps, core_ids=list(range(NCORES)), trace=trace)
    if trace:
        print(f"HW exec time: {res.exec_time_ns} ns")
        try:
            import pickle
            insts, tpath = res.instructions_and_trace
            rows = [(i.engine, i.name,
                     f"{getattr(i, 'source_line', '')}", i.timestamp,
                     i.duration) for i in insts]
            with open("/root/problem/work/insts.pkl", "wb") as f:
                pickle.dump({"rows": rows, "trace_path": tpath,
                             "exec_time_ns": res.exec_time_ns}, f)
            print(f"trace dumped: {len(rows)} insts, {tpath}")
        except Exception as e:
            print("trace dump failed:", e)

    out = np.stack([np.asarray(res.results[c]["out"]).reshape(SPC, L, D)
                    for c in range(NCORES)])
    return out.reshape(B, L, D).astype(np.float32)
